# revision 29
# baseline (speedup 1.0000x reference)
"""GQA attention kernel for Trainium2: B=1, S=4096, D=1024, H=8 heads (hd=128).

Sharding: one head per NeuronCore (8 cores). Each core computes its head's
Q/K/V projections from the (host-transposed, fp16) full hidden states, then a
causal flash-style attention entirely on-chip, writing its context slice
TRANSPOSED as [hd, S]; the host transposes back and concatenates heads.

Design (v2, fp16 pipeline):
  - All matmul inputs fp16 (1 cycle/row on PE, same as bf16/f32r), PSUM fp32.
  - Projections and attention are fused chunk-wise: attention for q-chunk n
    needs only K/V blocks 0..4n+3, which are ready after projection chunk n.
  - scoresT tiles [k=128(part), q<=512] batched two k-blocks per PSUM group
    [128,1024] so one ACT exp covers both; diagonal blocks use narrowed
    widths (512/384/256/128) packed into [128,896]+[128,384] groups.
  - Causal mask: multiplicative [128,128] triangle on the leading 128
    columns of each diagonal piece only.
  - Softmax denominator: exp partials accumulated on DVE in fp16 (2x mode)
    into zW [128,1024] (even|odd k-blocks), folded once, then one PE
    ones-matmul per q-chunk broadcasts the partition-sum; reciprocal via
    DVE reciprocal_approx_fast (fp32, ~2ULP@18bit); ctx normalized in fp32.
  - No output transpose on device: out is ctxT [hd, S] fp16; host upcasts
    and transposes.
"""

from contextlib import ExitStack

import numpy as np

B, S, D = 1, 4096, 1024
H = 8
HD = D // H  # 128
P = 128
QC = 512  # q-chunk (columns per scores tile)
NDC = D // P  # 8 d-chunks
NQC = S // QC  # 8 q-chunks
SCALE = 1.0 / float(np.sqrt(HD))


def _build_consts() -> np.ndarray:
    # [:, 0:128]  triangle: tri[kl, c] = 1.0 if kl <= c else 0.0
    # [:, 128:256] ones (lhsT for the Z broadcast matmul)
    kl = np.arange(P)[:, None]
    c = np.arange(P)[None, :]
    tri = (kl <= c).astype(np.float16)
    ones = np.ones((P, P), dtype=np.float16)
    return np.concatenate([tri, ones], axis=1)


def _build_program():
    nc = _build_program_inner()
    nc.finalize()
    return nc


def _build_program_inner():
    from concourse import bacc, mybir, tile
    from concourse.masks import make_identity

    f32 = mybir.dt.float32
    f16 = mybir.dt.float16
    Exp = mybir.ActivationFunctionType.Exp

    nc = bacc.Bacc("TRN2", target_bir_lowering=False, debug=False)

    xt = nc.dram_tensor("xt", [D, S], f16, kind="ExternalInput")
    wqt = nc.dram_tensor("wqt", [D, HD], f16, kind="ExternalInput")
    wkt = nc.dram_tensor("wkt", [D, HD], f16, kind="ExternalInput")
    wvt = nc.dram_tensor("wvt", [D, HD], f16, kind="ExternalInput")
    consts = nc.dram_tensor("consts", [P, 2 * P], f16, kind="ExternalInput")
    out = nc.dram_tensor("out", [HD, S], f16, kind="ExternalOutput")

    with ExitStack() as stack:
        tc = stack.enter_context(tile.TileContext(nc))
        constp = stack.enter_context(tc.tile_pool(name="const", bufs=1))
        wp = stack.enter_context(tc.tile_pool(name="w", bufs=1))
        xtp = stack.enter_context(tc.tile_pool(name="xts", bufs=2))
        qkp = stack.enter_context(tc.tile_pool(name="qk", bufs=8))
        vtp_sb = stack.enter_context(tc.tile_pool(name="vts", bufs=2))
        vnp = stack.enter_context(tc.tile_pool(name="vn", bufs=32))
        ep = stack.enter_context(tc.tile_pool(name="e", bufs=10))
        zp = stack.enter_context(tc.tile_pool(name="z", bufs=2))
        finp = stack.enter_context(tc.tile_pool(name="fin", bufs=2))
        pp = stack.enter_context(tc.tile_pool(name="ps", bufs=2, space="PSUM"))

        ident = constp.tile([P, P], f16, tag="ident")
        make_identity(nc, ident[:])
        csts = constp.tile([P, 2 * P], f16, tag="csts")
        tri = csts[:, 0:P]
        ones16 = csts[:, P:2 * P]

        # Startup ordering: first d-chunk of x first, then Wq, so the first
        # projection matmuls can start as early as possible.
        xt0 = xtp.tile([P, NDC * QC], f16, tag="xtn", name="xt0")
        nc.sync.dma_start(out=xt0[:, 0:QC], in_=xt[0:P, 0:QC])
        w_sb = {}
        for name, dram in (("q", wqt), ("k", wkt), ("v", wvt)):
            w_sb[name] = wp.tile([P, NDC * HD], f16, tag=f"w{name}", name=f"w{name}")
        nc.sync.dma_start(out=w_sb["q"][:, 0:HD], in_=wqt[0:P, :])
        nc.sync.dma_start(
            out=xt0[:].rearrange("p (a s) -> p a s", a=NDC)[:, 1:4, :],
            in_=xt[:, :].rearrange("(a p) s -> p a s", p=P)[:, 1:4, 0:QC],
        )
        nc.sync.dma_start(
            out=w_sb["q"][:].rearrange("p (a h) -> p a h", a=NDC)[:, 1:NDC, :],
            in_=wqt[:, :].rearrange("(a p) h -> p a h", p=P)[:, 1:NDC, :],
        )
        nc.sync.dma_start(
            out=xt0[:].rearrange("p (a s) -> p a s", a=NDC)[:, 4:NDC, :],
            in_=xt[:, :].rearrange("(a p) s -> p a s", p=P)[:, 4:NDC, 0:QC],
        )
        for name, dram in (("k", wkt), ("v", wvt)):
            nc.sync.dma_start(
                out=w_sb[name][:].rearrange("p (a h) -> p a h", a=NDC),
                in_=dram[:, :].rearrange("(a p) h -> p a h", p=P),
            )
        nc.sync.dma_start(out=csts[:], in_=consts[:, :])

        qt = [None] * NQC  # [hd, 512] per chunk, fp16
        kt = [None] * NQC
        vn = [None] * (4 * NQC)  # [k,hd] natural blocks, fp16

        with nc.allow_low_precision("fp16 attention pipeline"):
            for n in range(NQC):
                # ---- projections for chunk n (columns n*512..) ----
                if n == 0:
                    xt_n = xt0
                else:
                    xt_n = xtp.tile([P, NDC * QC], f16, tag="xtn")
                    nc.sync.dma_start(
                        out=xt_n[:].rearrange("p (a s) -> p a s", a=NDC),
                        in_=xt[:, :].rearrange(
                            "(a p) s -> p a s", p=P
                        )[:, :, n * QC:(n + 1) * QC],
                    )
                qt[n] = qkp.tile([P, QC], f16, tag="qt", name=f"qt{n}")
                kt[n] = qkp.tile([P, QC], f16, tag="kt", name=f"kt{n}")
                vt_n = vtp_sb.tile([P, QC], f16, tag="vt")
                for name, dst in (("q", qt[n]), ("k", kt[n]), ("v", vt_n)):
                    ps = pp.tile([P, QC], f32, tag="proj")
                    for d in range(NDC):
                        nc.tensor.matmul(
                            out=ps[:],
                            lhsT=w_sb[name][:, d * HD:(d + 1) * HD],
                            rhs=xt_n[:, d * QC:(d + 1) * QC],
                            start=(d == 0),
                            stop=(d == NDC - 1),
                        )
                    nc.vector.tensor_copy(out=dst[:], in_=ps[:])
                # V natural blocks via PE transpose
                for j in range(4):
                    pt = pp.tile([P, P], f16, tag="proj")
                    nc.tensor.transpose(
                        out=pt[:], in_=vt_n[:, j * P:(j + 1) * P], identity=ident[:]
                    )
                    vn[4 * n + j] = vnp.tile(
                        [P, P], f16, tag="vn", name=f"vn{4 * n + j}"
                    )
                    nc.vector.tensor_copy(out=vn[4 * n + j][:], in_=pt[:])

                # ---- attention for q-chunk qc = n ----
                qc = n
                nfull = 4 * qc  # full (non-diagonal) k-blocks
                q_rhs = qt[qc]
                c_ps = pp.tile([P, QC], f32, tag="ctx")
                zW = zp.tile([P, 2 * QC], f16, tag="zw")

                def kblk(ki):
                    return kt[ki // 4][:, (ki % 4) * P:(ki % 4 + 1) * P]

                # full k-block pairs
                for g in range(nfull // 2):
                    eg_ps = pp.tile([P, 2 * QC], f32, tag="eg")
                    for half in range(2):
                        nc.tensor.matmul(
                            out=eg_ps[:, half * QC:(half + 1) * QC],
                            lhsT=kblk(2 * g + half),
                            rhs=q_rhs[:],
                            start=True,
                            stop=True,
                            skip_group_check=True,
                        )
                    e = ep.tile([P, 2 * QC], f16, tag="e")
                    nc.scalar.activation(out=e[:], in_=eg_ps[:], func=Exp, scale=SCALE)
                    if g == 0:
                        nc.vector.tensor_copy(out=zW[:], in_=e[:])
                    else:
                        nc.vector.tensor_add(out=zW[:], in0=zW[:], in1=e[:])
                    for half in range(2):
                        nc.tensor.matmul(
                            out=c_ps[:],
                            lhsT=vn[2 * g + half][:],
                            rhs=e[:, half * QC:(half + 1) * QC],
                            start=(g == 0 and half == 0),
                            stop=False,
                            skip_group_check=True,
                        )

                # diagonal blocks 4qc+j, narrowed to width 512-128j, packed
                # [j0 512 | j1 384] and [j2 256 | j3 128]
                widths = [QC - P * j for j in range(4)]  # 512,384,256,128
                packs = [(0, 1), (2, 3)]
                e_diag = []
                for pi, (ja, jb) in enumerate(packs):
                    wa, wb = widths[ja], widths[jb]
                    d_ps = pp.tile([P, wa + wb], f32, tag="eg")
                    nc.tensor.matmul(
                        out=d_ps[:, 0:wa],
                        lhsT=kblk(nfull + ja),
                        rhs=q_rhs[:, P * ja:QC],
                        start=True, stop=True, skip_group_check=True,
                    )
                    nc.tensor.matmul(
                        out=d_ps[:, wa:wa + wb],
                        lhsT=kblk(nfull + jb),
                        rhs=q_rhs[:, P * jb:QC],
                        start=True, stop=True, skip_group_check=True,
                    )
                    ed = ep.tile([P, wa + wb], f16, tag=f"ed{pi}")
                    nc.scalar.activation(out=ed[:], in_=d_ps[:], func=Exp, scale=SCALE)
                    # triangle mask on the leading 128 columns of each piece
                    nc.vector.tensor_mul(out=ed[:, 0:P], in0=ed[:, 0:P], in1=tri)
                    nc.vector.tensor_mul(
                        out=ed[:, wa:wa + P], in0=ed[:, wa:wa + P], in1=tri
                    )
                    e_diag.append(ed)

                # z partial-sum adds for diagonal pieces. Piece j covers
                # q-columns [128j, 512). Even pieces go to the left half of
                # zW, odd pieces to the right half (left half when qc==0,
                # where the right half was never initialized).
                for pi, (ja, jb) in enumerate(packs):
                    wa, wb = widths[ja], widths[jb]
                    ed = e_diag[pi]
                    if qc == 0 and pi == 0:
                        nc.vector.tensor_copy(out=zW[:, 0:QC], in_=ed[:, 0:wa])
                    else:
                        nc.vector.tensor_add(
                            out=zW[:, P * ja:QC],
                            in0=zW[:, P * ja:QC],
                            in1=ed[:, 0:wa],
                        )
                    roff = 0 if qc == 0 else QC
                    nc.vector.tensor_add(
                        out=zW[:, roff + P * jb:roff + QC],
                        in0=zW[:, roff + P * jb:roff + QC],
                        in1=ed[:, wa:wa + wb],
                    )
                    # ctx accumulation for the two pieces
                    nc.tensor.matmul(
                        out=c_ps[:, P * ja:QC],
                        lhsT=vn[nfull + ja][:],
                        rhs=ed[:, 0:wa],
                        start=(nfull == 0 and pi == 0),
                        stop=False,
                        skip_group_check=True,
                    )
                    nc.tensor.matmul(
                        out=c_ps[:, P * jb:QC],
                        lhsT=vn[nfull + jb][:],
                        rhs=ed[:, wa:wa + wb],
                        start=False,
                        stop=(pi == 1),
                        skip_group_check=True,
                    )

                # fold zW halves, broadcast partition-sum via ones-matmul
                if qc == 0:
                    z_rhs = zW[:, 0:QC]
                else:
                    zfin = zp.tile([P, QC], f16, tag="zfin")
                    nc.vector.tensor_add(
                        out=zfin[:], in0=zW[:, 0:QC], in1=zW[:, QC:2 * QC]
                    )
                    z_rhs = zfin[:]
                zb_ps = pp.tile([P, QC], f32, tag="eg")
                rz = finp.tile([P, QC], f32, tag="rz")
                cs = finp.tile([P, QC], f16, tag="cs")
                if qc == NQC - 1:
                    # Last chunk: the z-broadcast -> reciprocal -> normalize
                    # -> DMA chain is the kernel's critical tail; pipeline it
                    # in two 256-column halves.
                    for h2 in range(2):
                        sl = slice(h2 * 256, (h2 + 1) * 256)
                        nc.tensor.matmul(
                            out=zb_ps[:, sl], lhsT=ones16, rhs=z_rhs[:, sl],
                            start=True, stop=True, skip_group_check=True,
                        )
                        nc.vector.reciprocal_approx_fast(
                            out=rz[:, sl], in_=zb_ps[:, sl]
                        )
                        nc.vector.tensor_mul(
                            out=cs[:, sl], in0=c_ps[:, sl], in1=rz[:, sl]
                        )
                        nc.sync.dma_start(
                            out=out[:, qc * QC + h2 * 256:qc * QC + (h2 + 1) * 256],
                            in_=cs[:, sl],
                        )
                else:
                    nc.tensor.matmul(
                        out=zb_ps[:], lhsT=ones16, rhs=z_rhs,
                        start=True, stop=True, skip_group_check=True,
                    )
                    nc.vector.reciprocal_approx_fast(out=rz[:], in_=zb_ps[:])
                    nc.vector.tensor_mul(out=cs[:], in0=c_ps[:], in1=rz[:])
                    nc.sync.dma_start(out=out[:, qc * QC:(qc + 1) * QC], in_=cs[:])

    return nc


_NC_CACHE = None


def _get_nc():
    global _NC_CACHE
    if _NC_CACHE is None:
        _NC_CACHE = _build_program()
    return _NC_CACHE


def kernel(hidden_states, Wq, Wk, Wv, trace=False, **trace_kwargs):
    from concourse.bass_utils import run_bass_kernel_spmd

    x = np.asarray(hidden_states, dtype=np.float32)[0]  # [S, D]
    xt = np.ascontiguousarray(x.T.astype(np.float16))  # [D, S]
    consts = _build_consts()
    wq = np.asarray(Wq, dtype=np.float32)
    wk = np.asarray(Wk, dtype=np.float32)
    wv = np.asarray(Wv, dtype=np.float32)
    in_maps = []
    for h in range(H):
        sl = slice(h * HD, (h + 1) * HD)
        in_maps.append({
            "xt": xt,
            "wqt": np.ascontiguousarray(wq[sl, :].T.astype(np.float16)),
            "wkt": np.ascontiguousarray(wk[sl, :].T.astype(np.float16)),
            "wvt": np.ascontiguousarray(wv[sl, :].T.astype(np.float16)),
            "consts": consts,
        })

    nc = _get_nc()
    res = run_bass_kernel_spmd(
        nc, in_maps, core_ids=list(range(H)), trace=trace, **trace_kwargs
    )
    ctx = np.empty((B, S, D), dtype=np.float32)
    for h in range(H):
        ctx[0, :, h * HD:(h + 1) * HD] = res.results[h]["out"].T.astype(np.float32)
    if trace:
        return ctx, res
    return ctx


# revision 30
# speedup vs baseline: 1.1782x; 1.1782x over previous
"""GQA attention kernel for Trainium2: B=1, S=4096, D=1024, H=8 heads (hd=128).

Sharding: one head per NeuronCore (8 cores). Each core computes its head's
Q/K/V projections from the (host-transposed, fp16) full hidden states, then a
causal flash-style attention entirely on-chip, writing its context slice
TRANSPOSED as [hd, S]; the host transposes back and concatenates heads.

Design (v2, fp16 pipeline):
  - All matmul inputs fp16 (1 cycle/row on PE, same as bf16/f32r), PSUM fp32.
  - Projections and attention are fused chunk-wise: attention for q-chunk n
    needs only K/V blocks 0..4n+3, which are ready after projection chunk n.
  - scoresT tiles [k=128(part), q<=512] batched two k-blocks per PSUM group
    [128,1024] so one ACT exp covers both; diagonal blocks use narrowed
    widths (512/384/256/128) packed into [128,896]+[128,384] groups.
  - Causal mask: multiplicative [128,128] triangle on the leading 128
    columns of each diagonal piece only.
  - Softmax denominator: exp partials accumulated on DVE in fp16 (2x mode)
    into zW [128,1024] (even|odd k-blocks), folded once, then one PE
    ones-matmul per q-chunk broadcasts the partition-sum; reciprocal via
    DVE reciprocal_approx_fast (fp32, ~2ULP@18bit); ctx normalized in fp32.
  - No output transpose on device: out is ctxT [hd, S] fp16; host upcasts
    and transposes.
"""

from contextlib import ExitStack

import numpy as np

B, S, D = 1, 4096, 1024
H = 8
HD = D // H  # 128
P = 128
QC = 512  # q-chunk (columns per scores tile)
NDC = D // P  # 8 d-chunks
NQC = S // QC  # 8 q-chunks
SCALE = 1.0 / float(np.sqrt(HD))


def _build_consts() -> np.ndarray:
    # [:, 0:128]  triangle: tri[kl, c] = 1.0 if kl <= c else 0.0
    # [:, 128:256] ones (lhsT for the Z broadcast matmul)
    kl = np.arange(P)[:, None]
    c = np.arange(P)[None, :]
    tri = (kl <= c).astype(np.float16)
    ones = np.ones((P, P), dtype=np.float16)
    return np.concatenate([tri, ones], axis=1)


def _build_program():
    nc = _build_program_inner()
    nc.finalize()
    return nc


def _build_program_inner():
    from concourse import bacc, mybir, tile
    from concourse.masks import make_identity

    f32 = mybir.dt.float32
    f16 = mybir.dt.float16
    Exp = mybir.ActivationFunctionType.Exp

    nc = bacc.Bacc("TRN2", target_bir_lowering=False, debug=False)

    xt = nc.dram_tensor("xt", [D, S], f16, kind="ExternalInput")
    wqt = nc.dram_tensor("wqt", [D, HD], f16, kind="ExternalInput")
    wkt = nc.dram_tensor("wkt", [D, HD], f16, kind="ExternalInput")
    wvt = nc.dram_tensor("wvt", [D, HD], f16, kind="ExternalInput")
    consts = nc.dram_tensor("consts", [P, 2 * P], f16, kind="ExternalInput")
    out = nc.dram_tensor("out", [HD, S], f16, kind="ExternalOutput")

    with ExitStack() as stack:
        tc = stack.enter_context(tile.TileContext(nc))
        constp = stack.enter_context(tc.tile_pool(name="const", bufs=1))
        wp = stack.enter_context(tc.tile_pool(name="w", bufs=1))
        xtp = stack.enter_context(tc.tile_pool(name="xts", bufs=2))
        qkp = stack.enter_context(tc.tile_pool(name="qk", bufs=8))
        vtp_sb = stack.enter_context(tc.tile_pool(name="vts", bufs=2))
        vnp = stack.enter_context(tc.tile_pool(name="vn", bufs=32))
        ep = stack.enter_context(tc.tile_pool(name="e", bufs=10))
        zp = stack.enter_context(tc.tile_pool(name="z", bufs=2))
        finp = stack.enter_context(tc.tile_pool(name="fin", bufs=2))
        pp = stack.enter_context(tc.tile_pool(name="ps", bufs=2, space="PSUM"))

        ident = constp.tile([P, P], f16, tag="ident")
        make_identity(nc, ident[:])
        csts = constp.tile([P, 2 * P], f16, tag="csts")
        tri = csts[:, 0:P]
        ones16 = csts[:, P:2 * P]

        # Startup ordering: first d-chunk of x first, then Wq, so the first
        # projection matmuls can start as early as possible.
        xt0 = xtp.tile([P, NDC * QC], f16, tag="xtn", name="xt0")
        nc.sync.dma_start(out=xt0[:, 0:QC], in_=xt[0:P, 0:QC])
        w_sb = {}
        for name, dram in (("q", wqt), ("k", wkt), ("v", wvt)):
            w_sb[name] = wp.tile([P, NDC * HD], f16, tag=f"w{name}", name=f"w{name}")
        nc.sync.dma_start(
            out=w_sb["q"][:].rearrange("p (a h) -> p a h", a=NDC),
            in_=wqt[:, :].rearrange("(a p) h -> p a h", p=P),
        )
        nc.sync.dma_start(
            out=xt0[:].rearrange("p (a s) -> p a s", a=NDC)[:, 1:NDC, :],
            in_=xt[:, :].rearrange("(a p) s -> p a s", p=P)[:, 1:NDC, 0:QC],
        )
        for name, dram in (("k", wkt), ("v", wvt)):
            nc.sync.dma_start(
                out=w_sb[name][:].rearrange("p (a h) -> p a h", a=NDC),
                in_=dram[:, :].rearrange("(a p) h -> p a h", p=P),
            )
        nc.sync.dma_start(out=csts[:], in_=consts[:, :])

        qt = [None] * NQC  # [hd, 512] per chunk, fp16
        kt = [None] * NQC
        vn = [None] * (4 * NQC)  # [k,hd] natural blocks, fp16

        with nc.allow_low_precision("fp16 attention pipeline"):
            for n in range(NQC):
                # ---- projections for chunk n (columns n*512..) ----
                if n == 0:
                    xt_n = xt0
                else:
                    xt_n = xtp.tile([P, NDC * QC], f16, tag="xtn")
                    nc.sync.dma_start(
                        out=xt_n[:].rearrange("p (a s) -> p a s", a=NDC),
                        in_=xt[:, :].rearrange(
                            "(a p) s -> p a s", p=P
                        )[:, :, n * QC:(n + 1) * QC],
                    )
                qt[n] = qkp.tile([P, QC], f16, tag="qt", name=f"qt{n}")
                kt[n] = qkp.tile([P, QC], f16, tag="kt", name=f"kt{n}")
                vt_n = vtp_sb.tile([P, QC], f16, tag="vt")
                for name, dst in (("q", qt[n]), ("k", kt[n]), ("v", vt_n)):
                    ps = pp.tile([P, QC], f32, tag="proj")
                    for d in range(NDC):
                        nc.tensor.matmul(
                            out=ps[:],
                            lhsT=w_sb[name][:, d * HD:(d + 1) * HD],
                            rhs=xt_n[:, d * QC:(d + 1) * QC],
                            start=(d == 0),
                            stop=(d == NDC - 1),
                        )
                    nc.vector.tensor_copy(out=dst[:], in_=ps[:])
                # V natural blocks via PE transpose
                for j in range(4):
                    pt = pp.tile([P, P], f16, tag="proj")
                    nc.tensor.transpose(
                        out=pt[:], in_=vt_n[:, j * P:(j + 1) * P], identity=ident[:]
                    )
                    vn[4 * n + j] = vnp.tile(
                        [P, P], f16, tag="vn", name=f"vn{4 * n + j}"
                    )
                    nc.vector.tensor_copy(out=vn[4 * n + j][:], in_=pt[:])

                # ---- attention for q-chunk qc = n ----
                qc = n
                nfull = 4 * qc  # full (non-diagonal) k-blocks
                q_rhs = qt[qc]
                c_ps = pp.tile([P, QC], f32, tag="ctx")
                zW = zp.tile([P, 2 * QC], f16, tag="zw")

                def kblk(ki):
                    return kt[ki // 4][:, (ki % 4) * P:(ki % 4 + 1) * P]

                # full k-block pairs
                for g in range(nfull // 2):
                    eg_ps = pp.tile([P, 2 * QC], f32, tag="eg")
                    for half in range(2):
                        nc.tensor.matmul(
                            out=eg_ps[:, half * QC:(half + 1) * QC],
                            lhsT=kblk(2 * g + half),
                            rhs=q_rhs[:],
                            start=True,
                            stop=True,
                            skip_group_check=True,
                        )
                    e = ep.tile([P, 2 * QC], f16, tag="e")
                    nc.scalar.activation(out=e[:], in_=eg_ps[:], func=Exp, scale=SCALE)
                    if g == 0:
                        nc.vector.tensor_copy(out=zW[:], in_=e[:])
                    else:
                        nc.vector.tensor_add(out=zW[:], in0=zW[:], in1=e[:])
                    for half in range(2):
                        nc.tensor.matmul(
                            out=c_ps[:],
                            lhsT=vn[2 * g + half][:],
                            rhs=e[:, half * QC:(half + 1) * QC],
                            start=(g == 0 and half == 0),
                            stop=False,
                            skip_group_check=True,
                        )

                # diagonal blocks 4qc+j, narrowed to width 512-128j, packed
                # [j0 512 | j1 384] and [j2 256 | j3 128]
                widths = [QC - P * j for j in range(4)]  # 512,384,256,128
                packs = [(0, 1), (2, 3)]
                e_diag = []
                for pi, (ja, jb) in enumerate(packs):
                    wa, wb = widths[ja], widths[jb]
                    d_ps = pp.tile([P, wa + wb], f32, tag="eg")
                    nc.tensor.matmul(
                        out=d_ps[:, 0:wa],
                        lhsT=kblk(nfull + ja),
                        rhs=q_rhs[:, P * ja:QC],
                        start=True, stop=True, skip_group_check=True,
                    )
                    nc.tensor.matmul(
                        out=d_ps[:, wa:wa + wb],
                        lhsT=kblk(nfull + jb),
                        rhs=q_rhs[:, P * jb:QC],
                        start=True, stop=True, skip_group_check=True,
                    )
                    ed = ep.tile([P, wa + wb], f16, tag=f"ed{pi}")
                    nc.scalar.activation(out=ed[:], in_=d_ps[:], func=Exp, scale=SCALE)
                    # triangle mask on the leading 128 columns of each piece
                    nc.vector.tensor_mul(out=ed[:, 0:P], in0=ed[:, 0:P], in1=tri)
                    nc.vector.tensor_mul(
                        out=ed[:, wa:wa + P], in0=ed[:, wa:wa + P], in1=tri
                    )
                    e_diag.append(ed)

                # z partial-sum adds for diagonal pieces. Piece j covers
                # q-columns [128j, 512). Even pieces go to the left half of
                # zW, odd pieces to the right half (left half when qc==0,
                # where the right half was never initialized).
                for pi, (ja, jb) in enumerate(packs):
                    wa, wb = widths[ja], widths[jb]
                    ed = e_diag[pi]
                    if qc == 0 and pi == 0:
                        nc.vector.tensor_copy(out=zW[:, 0:QC], in_=ed[:, 0:wa])
                    else:
                        nc.vector.tensor_add(
                            out=zW[:, P * ja:QC],
                            in0=zW[:, P * ja:QC],
                            in1=ed[:, 0:wa],
                        )
                    roff = 0 if qc == 0 else QC
                    nc.vector.tensor_add(
                        out=zW[:, roff + P * jb:roff + QC],
                        in0=zW[:, roff + P * jb:roff + QC],
                        in1=ed[:, wa:wa + wb],
                    )
                    # ctx accumulation for the two pieces
                    nc.tensor.matmul(
                        out=c_ps[:, P * ja:QC],
                        lhsT=vn[nfull + ja][:],
                        rhs=ed[:, 0:wa],
                        start=(nfull == 0 and pi == 0),
                        stop=False,
                        skip_group_check=True,
                    )
                    nc.tensor.matmul(
                        out=c_ps[:, P * jb:QC],
                        lhsT=vn[nfull + jb][:],
                        rhs=ed[:, wa:wa + wb],
                        start=False,
                        stop=(pi == 1),
                        skip_group_check=True,
                    )

                # fold zW halves, broadcast partition-sum via ones-matmul
                if qc == 0:
                    z_rhs = zW[:, 0:QC]
                else:
                    zfin = zp.tile([P, QC], f16, tag="zfin")
                    nc.vector.tensor_add(
                        out=zfin[:], in0=zW[:, 0:QC], in1=zW[:, QC:2 * QC]
                    )
                    z_rhs = zfin[:]
                zb_ps = pp.tile([P, QC], f32, tag="eg")
                nc.tensor.matmul(
                    out=zb_ps[:], lhsT=ones16, rhs=z_rhs,
                    start=True, stop=True, skip_group_check=True,
                )
                rz = finp.tile([P, QC], f32, tag="rz")
                nc.vector.reciprocal_approx_fast(out=rz[:], in_=zb_ps[:])
                cs = finp.tile([P, QC], f16, tag="cs")
                nc.vector.tensor_mul(out=cs[:], in0=c_ps[:], in1=rz[:])
                nc.sync.dma_start(out=out[:, qc * QC:(qc + 1) * QC], in_=cs[:])

    return nc


_NC_CACHE = None


def _get_nc():
    global _NC_CACHE
    if _NC_CACHE is None:
        _NC_CACHE = _build_program()
    return _NC_CACHE


def kernel(hidden_states, Wq, Wk, Wv, trace=False, **trace_kwargs):
    from concourse.bass_utils import run_bass_kernel_spmd

    x = np.asarray(hidden_states, dtype=np.float32)[0]  # [S, D]
    xt = np.ascontiguousarray(x.T.astype(np.float16))  # [D, S]
    consts = _build_consts()
    wq = np.asarray(Wq, dtype=np.float32)
    wk = np.asarray(Wk, dtype=np.float32)
    wv = np.asarray(Wv, dtype=np.float32)
    in_maps = []
    for h in range(H):
        sl = slice(h * HD, (h + 1) * HD)
        in_maps.append({
            "xt": xt,
            "wqt": np.ascontiguousarray(wq[sl, :].T.astype(np.float16)),
            "wkt": np.ascontiguousarray(wk[sl, :].T.astype(np.float16)),
            "wvt": np.ascontiguousarray(wv[sl, :].T.astype(np.float16)),
            "consts": consts,
        })

    nc = _get_nc()
    res = run_bass_kernel_spmd(
        nc, in_maps, core_ids=list(range(H)), trace=trace, **trace_kwargs
    )
    ctx = np.empty((B, S, D), dtype=np.float32)
    for h in range(H):
        ctx[0, :, h * HD:(h + 1) * HD] = res.results[h]["out"].T.astype(np.float32)
    if trace:
        return ctx, res
    return ctx


# revision 31
# speedup vs baseline: 1.1844x; 1.0052x over previous
"""GQA attention kernel for Trainium2: B=1, S=4096, D=1024, H=8 heads (hd=128).

Sharding: one head per NeuronCore (8 cores). Each core computes its head's
Q/K/V projections from the (host-transposed, fp16) full hidden states, then a
causal flash-style attention entirely on-chip, writing its context slice
TRANSPOSED as [hd, S]; the host transposes back and concatenates heads.

Design (v2, fp16 pipeline):
  - All matmul inputs fp16 (1 cycle/row on PE, same as bf16/f32r), PSUM fp32.
  - Projections and attention are fused chunk-wise: attention for q-chunk n
    needs only K/V blocks 0..4n+3, which are ready after projection chunk n.
  - scoresT tiles [k=128(part), q<=512] batched two k-blocks per PSUM group
    [128,1024] so one ACT exp covers both; diagonal blocks use narrowed
    widths (512/384/256/128) packed into [128,896]+[128,384] groups.
  - Causal mask: multiplicative [128,128] triangle on the leading 128
    columns of each diagonal piece only.
  - Softmax denominator: exp partials accumulated on DVE in fp16 (2x mode)
    into zW [128,1024] (even|odd k-blocks), folded once, then one PE
    ones-matmul per q-chunk broadcasts the partition-sum; reciprocal via
    DVE reciprocal_approx_fast (fp32, ~2ULP@18bit); ctx normalized in fp32.
  - No output transpose on device: out is ctxT [hd, S] fp16; host upcasts
    and transposes.
"""

from contextlib import ExitStack

import numpy as np

B, S, D = 1, 4096, 1024
H = 8
HD = D // H  # 128
P = 128
QC = 512  # q-chunk (columns per scores tile)
NDC = D // P  # 8 d-chunks
NQC = S // QC  # 8 q-chunks
SCALE = 1.0 / float(np.sqrt(HD))


def _build_consts() -> np.ndarray:
    # [:, 0:128]  triangle: tri[kl, c] = 1.0 if kl <= c else 0.0
    # [:, 128:256] ones (lhsT for the Z broadcast matmul)
    kl = np.arange(P)[:, None]
    c = np.arange(P)[None, :]
    tri = (kl <= c).astype(np.float16)
    ones = np.ones((P, P), dtype=np.float16)
    return np.concatenate([tri, ones], axis=1)


def _build_program():
    nc = _build_program_inner()
    nc.finalize()
    return nc


def _build_program_inner():
    from concourse import bacc, mybir, tile
    from concourse.masks import make_identity

    f32 = mybir.dt.float32
    f16 = mybir.dt.float16
    Exp = mybir.ActivationFunctionType.Exp

    nc = bacc.Bacc("TRN2", target_bir_lowering=False, debug=False)

    xt = nc.dram_tensor("xt", [D, S], f16, kind="ExternalInput")
    wqt = nc.dram_tensor("wqt", [D, HD], f16, kind="ExternalInput")
    wkt = nc.dram_tensor("wkt", [D, HD], f16, kind="ExternalInput")
    wvt = nc.dram_tensor("wvt", [D, HD], f16, kind="ExternalInput")
    consts = nc.dram_tensor("consts", [P, 2 * P], f16, kind="ExternalInput")
    out = nc.dram_tensor("out", [HD, S], f16, kind="ExternalOutput")

    with ExitStack() as stack:
        tc = stack.enter_context(tile.TileContext(nc))
        constp = stack.enter_context(tc.tile_pool(name="const", bufs=1))
        wp = stack.enter_context(tc.tile_pool(name="w", bufs=1))
        xtp = stack.enter_context(tc.tile_pool(name="xts", bufs=2))
        qkp = stack.enter_context(tc.tile_pool(name="qk", bufs=8))
        vtp_sb = stack.enter_context(tc.tile_pool(name="vts", bufs=2))
        vnp = stack.enter_context(tc.tile_pool(name="vn", bufs=32))
        ep = stack.enter_context(tc.tile_pool(name="e", bufs=10))
        zp = stack.enter_context(tc.tile_pool(name="z", bufs=2))
        finp = stack.enter_context(tc.tile_pool(name="fin", bufs=2))
        pp = stack.enter_context(tc.tile_pool(name="ps", bufs=2, space="PSUM"))

        ident = constp.tile([P, P], f16, tag="ident")
        make_identity(nc, ident[:])
        csts = constp.tile([P, 2 * P], f16, tag="csts")
        tri = csts[:, 0:P]
        ones16 = csts[:, P:2 * P]

        # Startup ordering: first d-chunk of x first, then Wq, so the first
        # projection matmuls can start as early as possible.
        xt0 = xtp.tile([P, NDC * QC], f16, tag="xtn", name="xt0")
        nc.sync.dma_start(out=xt0[:, 0:QC], in_=xt[0:P, 0:QC])
        w_sb = {}
        for name, dram in (("q", wqt), ("k", wkt), ("v", wvt)):
            w_sb[name] = wp.tile([P, NDC * HD], f16, tag=f"w{name}", name=f"w{name}")
        nc.sync.dma_start(
            out=w_sb["q"][:].rearrange("p (a h) -> p a h", a=NDC),
            in_=wqt[:, :].rearrange("(a p) h -> p a h", p=P),
        )
        nc.sync.dma_start(
            out=xt0[:].rearrange("p (a s) -> p a s", a=NDC)[:, 1:NDC, :],
            in_=xt[:, :].rearrange("(a p) s -> p a s", p=P)[:, 1:NDC, 0:QC],
        )
        for name, dram in (("k", wkt), ("v", wvt)):
            nc.sync.dma_start(
                out=w_sb[name][:].rearrange("p (a h) -> p a h", a=NDC),
                in_=dram[:, :].rearrange("(a p) h -> p a h", p=P),
            )
        nc.sync.dma_start(out=csts[:], in_=consts[:, :])

        # PE clock warmup sized to the startup DMA window: HAM reaches full
        # clock after ~3us of continuous busy, and an idle gap resets it.
        # 44 throwaway matmuls (~160ns apiece) bridge ~7.4us -> ~14.3us,
        # when the bulk xt0/w transfers land, so real projections start at
        # full speed with no ramp reset.
        for wi in range(44):
            wps = pp.tile([P, P], f32, tag="proj", name=f"warm{wi}")
            nc.tensor.matmul(
                out=wps[:], lhsT=ident[:], rhs=ident[:],
                start=True, stop=True, skip_group_check=True,
            )

        qt = [None] * NQC  # [hd, 512] per chunk, fp16
        kt = [None] * NQC
        vn = [None] * (4 * NQC)  # [k,hd] natural blocks, fp16

        with nc.allow_low_precision("fp16 attention pipeline"):
            for n in range(NQC):
                # ---- projections for chunk n (columns n*512..) ----
                if n == 0:
                    xt_n = xt0
                else:
                    xt_n = xtp.tile([P, NDC * QC], f16, tag="xtn")
                    nc.sync.dma_start(
                        out=xt_n[:].rearrange("p (a s) -> p a s", a=NDC),
                        in_=xt[:, :].rearrange(
                            "(a p) s -> p a s", p=P
                        )[:, :, n * QC:(n + 1) * QC],
                    )
                qt[n] = qkp.tile([P, QC], f16, tag="qt", name=f"qt{n}")
                kt[n] = qkp.tile([P, QC], f16, tag="kt", name=f"kt{n}")
                vt_n = vtp_sb.tile([P, QC], f16, tag="vt")
                for name, dst in (("q", qt[n]), ("k", kt[n]), ("v", vt_n)):
                    ps = pp.tile([P, QC], f32, tag="proj")
                    for d in range(NDC):
                        nc.tensor.matmul(
                            out=ps[:],
                            lhsT=w_sb[name][:, d * HD:(d + 1) * HD],
                            rhs=xt_n[:, d * QC:(d + 1) * QC],
                            start=(d == 0),
                            stop=(d == NDC - 1),
                        )
                    nc.vector.tensor_copy(out=dst[:], in_=ps[:])
                # V natural blocks via PE transpose
                for j in range(4):
                    pt = pp.tile([P, P], f16, tag="proj")
                    nc.tensor.transpose(
                        out=pt[:], in_=vt_n[:, j * P:(j + 1) * P], identity=ident[:]
                    )
                    vn[4 * n + j] = vnp.tile(
                        [P, P], f16, tag="vn", name=f"vn{4 * n + j}"
                    )
                    nc.vector.tensor_copy(out=vn[4 * n + j][:], in_=pt[:])

                # ---- attention for q-chunk qc = n ----
                qc = n
                nfull = 4 * qc  # full (non-diagonal) k-blocks
                q_rhs = qt[qc]
                c_ps = pp.tile([P, QC], f32, tag="ctx")
                zW = zp.tile([P, 2 * QC], f16, tag="zw")

                def kblk(ki):
                    return kt[ki // 4][:, (ki % 4) * P:(ki % 4 + 1) * P]

                # full k-block pairs
                for g in range(nfull // 2):
                    eg_ps = pp.tile([P, 2 * QC], f32, tag="eg")
                    for half in range(2):
                        nc.tensor.matmul(
                            out=eg_ps[:, half * QC:(half + 1) * QC],
                            lhsT=kblk(2 * g + half),
                            rhs=q_rhs[:],
                            start=True,
                            stop=True,
                            skip_group_check=True,
                        )
                    e = ep.tile([P, 2 * QC], f16, tag="e")
                    nc.scalar.activation(out=e[:], in_=eg_ps[:], func=Exp, scale=SCALE)
                    if g == 0:
                        nc.vector.tensor_copy(out=zW[:], in_=e[:])
                    else:
                        nc.vector.tensor_add(out=zW[:], in0=zW[:], in1=e[:])
                    for half in range(2):
                        nc.tensor.matmul(
                            out=c_ps[:],
                            lhsT=vn[2 * g + half][:],
                            rhs=e[:, half * QC:(half + 1) * QC],
                            start=(g == 0 and half == 0),
                            stop=False,
                            skip_group_check=True,
                        )

                # diagonal blocks 4qc+j, narrowed to width 512-128j, packed
                # [j0 512 | j1 384] and [j2 256 | j3 128]
                widths = [QC - P * j for j in range(4)]  # 512,384,256,128
                packs = [(0, 1), (2, 3)]
                e_diag = []
                for pi, (ja, jb) in enumerate(packs):
                    wa, wb = widths[ja], widths[jb]
                    d_ps = pp.tile([P, wa + wb], f32, tag="eg")
                    nc.tensor.matmul(
                        out=d_ps[:, 0:wa],
                        lhsT=kblk(nfull + ja),
                        rhs=q_rhs[:, P * ja:QC],
                        start=True, stop=True, skip_group_check=True,
                    )
                    nc.tensor.matmul(
                        out=d_ps[:, wa:wa + wb],
                        lhsT=kblk(nfull + jb),
                        rhs=q_rhs[:, P * jb:QC],
                        start=True, stop=True, skip_group_check=True,
                    )
                    ed = ep.tile([P, wa + wb], f16, tag=f"ed{pi}")
                    nc.scalar.activation(out=ed[:], in_=d_ps[:], func=Exp, scale=SCALE)
                    # triangle mask on the leading 128 columns of each piece
                    nc.vector.tensor_mul(out=ed[:, 0:P], in0=ed[:, 0:P], in1=tri)
                    nc.vector.tensor_mul(
                        out=ed[:, wa:wa + P], in0=ed[:, wa:wa + P], in1=tri
                    )
                    e_diag.append(ed)

                # z partial-sum adds for diagonal pieces. Piece j covers
                # q-columns [128j, 512). Even pieces go to the left half of
                # zW, odd pieces to the right half (left half when qc==0,
                # where the right half was never initialized).
                for pi, (ja, jb) in enumerate(packs):
                    wa, wb = widths[ja], widths[jb]
                    ed = e_diag[pi]
                    if qc == 0 and pi == 0:
                        nc.vector.tensor_copy(out=zW[:, 0:QC], in_=ed[:, 0:wa])
                    else:
                        nc.vector.tensor_add(
                            out=zW[:, P * ja:QC],
                            in0=zW[:, P * ja:QC],
                            in1=ed[:, 0:wa],
                        )
                    roff = 0 if qc == 0 else QC
                    nc.vector.tensor_add(
                        out=zW[:, roff + P * jb:roff + QC],
                        in0=zW[:, roff + P * jb:roff + QC],
                        in1=ed[:, wa:wa + wb],
                    )
                    # ctx accumulation for the two pieces
                    nc.tensor.matmul(
                        out=c_ps[:, P * ja:QC],
                        lhsT=vn[nfull + ja][:],
                        rhs=ed[:, 0:wa],
                        start=(nfull == 0 and pi == 0),
                        stop=False,
                        skip_group_check=True,
                    )
                    nc.tensor.matmul(
                        out=c_ps[:, P * jb:QC],
                        lhsT=vn[nfull + jb][:],
                        rhs=ed[:, wa:wa + wb],
                        start=False,
                        stop=(pi == 1),
                        skip_group_check=True,
                    )

                # fold zW halves, broadcast partition-sum via ones-matmul
                if qc == 0:
                    z_rhs = zW[:, 0:QC]
                else:
                    zfin = zp.tile([P, QC], f16, tag="zfin")
                    nc.vector.tensor_add(
                        out=zfin[:], in0=zW[:, 0:QC], in1=zW[:, QC:2 * QC]
                    )
                    z_rhs = zfin[:]
                zb_ps = pp.tile([P, QC], f32, tag="eg")
                nc.tensor.matmul(
                    out=zb_ps[:], lhsT=ones16, rhs=z_rhs,
                    start=True, stop=True, skip_group_check=True,
                )
                rz = finp.tile([P, QC], f32, tag="rz")
                nc.vector.reciprocal_approx_fast(out=rz[:], in_=zb_ps[:])
                cs = finp.tile([P, QC], f16, tag="cs")
                nc.vector.tensor_mul(out=cs[:], in0=c_ps[:], in1=rz[:])
                nc.sync.dma_start(out=out[:, qc * QC:(qc + 1) * QC], in_=cs[:])

    return nc


_NC_CACHE = None


def _get_nc():
    global _NC_CACHE
    if _NC_CACHE is None:
        _NC_CACHE = _build_program()
    return _NC_CACHE


def kernel(hidden_states, Wq, Wk, Wv, trace=False, **trace_kwargs):
    from concourse.bass_utils import run_bass_kernel_spmd

    x = np.asarray(hidden_states, dtype=np.float32)[0]  # [S, D]
    xt = np.ascontiguousarray(x.T.astype(np.float16))  # [D, S]
    consts = _build_consts()
    wq = np.asarray(Wq, dtype=np.float32)
    wk = np.asarray(Wk, dtype=np.float32)
    wv = np.asarray(Wv, dtype=np.float32)
    in_maps = []
    for h in range(H):
        sl = slice(h * HD, (h + 1) * HD)
        in_maps.append({
            "xt": xt,
            "wqt": np.ascontiguousarray(wq[sl, :].T.astype(np.float16)),
            "wkt": np.ascontiguousarray(wk[sl, :].T.astype(np.float16)),
            "wvt": np.ascontiguousarray(wv[sl, :].T.astype(np.float16)),
            "consts": consts,
        })

    nc = _get_nc()
    res = run_bass_kernel_spmd(
        nc, in_maps, core_ids=list(range(H)), trace=trace, **trace_kwargs
    )
    ctx = np.empty((B, S, D), dtype=np.float32)
    for h in range(H):
        ctx[0, :, h * HD:(h + 1) * HD] = res.results[h]["out"].T.astype(np.float32)
    if trace:
        return ctx, res
    return ctx
